# revision 48
# baseline (speedup 1.0000x reference)
"""Hadamard transform kernel for Trainium2 (8 NeuronCores, SPMD data parallel).

y = (1/48) * (H36 (x) H64) @ x_row  per token row, x: (4, 8192, 2304) fp32.

Math: view each row as X[j=36, c=64] (row-major).  Then
    y[k*64+m] = (1/48) * sum_j sum_c had_k[k,j] * H64[m,c] * X[j,c]
with H64 the natural-order Sylvester Hadamard (symmetric).

Device scheme (per 6-token "group", no on-chip transposes needed):
  mm1: lhsT = Xg[(t3,j)=108 part, (trip2,c)=128 free]   (x data as stationary)
       rhs  = W36 = blockdiag(had_k.T x3) [108,108]
       out  = Z[(trip2,c)=128, (t3,k)=108]  (PSUM fp32)
  mm2: lhsT = Z (cast bf16) [128, 108]
       rhs  = W64 = blockdiag(H64 x2) [128,128]
       out  = Y[(t3,k)=108, (trip2,m)=128]  (PSUM fp32)

HBM layout: the HOST pre-permutes x into xp[108, NG*128] (partition-major,
per-group contiguous) and un-permutes y afterwards.  On device every DMA is
then one 2-4KB contiguous run per partition -- ~40x fewer descriptors than
the strided token layout, which puts the DMA path at HBM line rate.
"""

import numpy as np

D = 2304
NTOK = 4096          # tokens per core
NCORES = 8
SB_G = 15            # groups per superblock: 15*256B = 3840B per partition
                     # per DMA descriptor -- fits one <=4KB SDMA packet
QUAD = 8             # groups per PSUM bank batch


def _h64():
    m, c = np.meshgrid(np.arange(64), np.arange(64), indexing="ij")
    bits = np.zeros((64, 64), np.int64)
    v = m & c
    for _ in range(6):
        bits += v & 1
        v >>= 1
    return np.where(bits % 2 == 0, 1.0, -1.0).astype(np.float32)


def _group_bases(ntok):
    ngfull = ntok // 6
    bases = [6 * g for g in range(ngfull)]
    if ntok % 6:
        bases.append(ntok - 6)  # overlap group, rewrites a few tokens identically
    return bases


def _build_program_raw(w36_np, w64_np, ng_total):
    from contextlib import ExitStack
    import concourse.bass as bass
    import concourse.mybir as mybir
    from concourse.bass_types import AP

    FREE = ng_total * 128
    nc = bass.Bass()
    x = nc.dram_tensor("x", [108, FREE], mybir.dt.bfloat16, kind="ExternalInput")
    y = nc.dram_tensor("y", [108, FREE], mybir.dt.bfloat16, kind="ExternalOutput")
    w36_d = nc.inline_tensor(w36_np, name="w36")
    w64_d = nc.inline_tensor(w64_np, name="w64")

    # superblocks: (first_group, n_groups)
    sbs = []
    g = 0
    while g < ng_total:
        n = min(SB_G, ng_total - g)
        sbs.append((g, n))
        g += n
    nsb = len(sbs)

    def dram_ap(t, g0, gcount):
        # one contiguous gcount*256B run per partition
        return AP(tensor=t, offset=g0 * 128,
                  ap=[[FREE, 108], [1, gcount * 128]])

    # quads: global list of (sb_idx, q0, nq)
    quads = []
    for si, (g0, ng) in enumerate(sbs):
        q0 = 0
        while q0 < ng:
            quads.append((si, q0, min(QUAD, ng - q0)))
            q0 += QUAD
    nquads = len(quads)
    first_quad = [0]
    for si, (g0, ng) in enumerate(sbs):
        first_quad.append(first_quad[-1] + (ng + QUAD - 1) // QUAD)

    NZ = 2   # zps/zsb ring depth
    NY = 2   # yps ring depth

    with ExitStack() as ctx:
        w36 = ctx.enter_context(nc.sbuf_tensor("w36sb", [108, 108], mybir.dt.bfloat16))
        w64 = ctx.enter_context(nc.sbuf_tensor("w64sb", [128, 128], mybir.dt.bfloat16))

        XR = 6   # xt ring: loads run several sbs ahead of the PE
        YR = 3
        xt = [ctx.enter_context(nc.sbuf_tensor(f"xt{i}", [108, SB_G, 128], mybir.dt.bfloat16)) for i in range(XR)]
        yt = [ctx.enter_context(nc.sbuf_tensor(f"yt{i}", [108, SB_G, 128], mybir.dt.bfloat16)) for i in range(YR)]
        # zsb/yps padded to 128 free columns / partitions: a full-128-column
        # stationary makes mm2's LDWEIGHTS eligible for fast weight load.
        # Columns 108:128 hold stale garbage that lands in PSUM partitions
        # 108:128, which nothing reads.
        # zps group stride padded to 128 (512B, bank-aligned) so no matmul
        # output tile straddles a 2KB PSUM bank boundary.
        zsb = [ctx.enter_context(nc.sbuf_tensor(f"zsb{i}", [128, QUAD, 128], mybir.dt.bfloat16)) for i in range(NZ)]
        zps = [ctx.enter_context(nc.psum_tensor(f"zps{i}", [128, QUAD, 128], mybir.dt.float32)) for i in range(NZ)]
        yps = [ctx.enter_context(nc.psum_tensor(f"yps{i}", [128, QUAD, 128], mybir.dt.float32)) for i in range(NY)]
        s_in = ctx.enter_context(nc.semaphore())
        s_pe1 = ctx.enter_context(nc.semaphore())
        s_act = ctx.enter_context(nc.semaphore())
        s_pe2 = ctx.enter_context(nc.semaphore())
        s_dve = ctx.enter_context(nc.semaphore())
        s_out = ctx.enter_context(nc.semaphore())
        s_w = ctx.enter_context(nc.semaphore())
        s_nil = ctx.enter_context(nc.semaphore())  # sink for data DMAs; never waited on
        blk = ctx.enter_context(nc.Block())

        @blk.gpsimd
        def _(g):
            # Loads AND stores share the SWDGE queue (spreads packets over
            # all 16 SDMA engines; HWDGE only uses 12 for this shape).
            # Stores trail the loads by 5 sbs in the issue stream so the
            # store's data-ready wait never delays a load that keeps the
            # PE fed.
            # A DMA's 16 sem incs are NOT strictly ordered after all of its
            # data descriptors when the transfer is only ~108 descriptors,
            # so consumers wait for the NEXT transfer's incs too (one sb of
            # slack >> the observed sub-us race window); a dummy trailing
            # load provides that slack for the final sb.
            SLAG = 5
            for it in range(nsb + SLAG + 1):
                if it < nsb:
                    g0, ng = sbs[it]
                    if it >= XR:  # xtile reuse: mm1s of sb it-XR done
                        g.wait_ge(s_pe1, first_quad[it - XR + 1])
                    g.dma_start(xt[it % XR][:, 0:ng, :],
                                dram_ap(x, g0, ng)).then_inc(s_in, 16)
                elif it == nsb:
                    g.wait_ge(s_pe1, first_quad[nsb - XR + 1])  # xt[nsb%XR] free
                    g.dma_start(xt[nsb % XR][:, 0:1, :],
                                dram_ap(x, 0, 1)).then_inc(s_in, 16)
                sj = it - SLAG
                if 0 <= sj < nsb:
                    g0, ng = sbs[sj]
                    g.wait_ge(s_dve, first_quad[sj + 1])
                    g.dma_start(dram_ap(y, g0, ng),
                                yt[sj % YR][:, 0:ng, :]).then_inc(s_out, 16)

        @blk.tensor
        def _(t):
            # Software-pipelined: mm1 of quad qi runs ahead of mm2 of quad
            # qi-1, so the scalar copy1(qi-1) overlaps mm1(qi) instead of
            # stalling the PE.
            t.wait_ge(s_w, 48)
            for qi in range(nquads + 1):
                if qi < nquads:
                    si, q0, nq = quads[qi]
                    if q0 == 0:
                        t.wait_ge(s_in, 16 * (si + 2))  # sb si + one-sb slack
                    # (zps[qi%NZ] free is implied: the previous iteration's
                    # mm2 wait saw s_act >= qi, and copy1 is monotone)
                    for q in range(nq):
                        i = nc.tensor.matmul(zps[qi % NZ][:, q, 0:108],
                                             xt[si % XR][:, q0 + q, :], w36[:, :],
                                             start=(q % 4 == 0),
                                             stop=(q % 4 == 3 or q == nq - 1))
                    i.then_inc(s_pe1, 1)
                if qi >= 1:
                    qj = qi - 1
                    _, _, nqj = quads[qj]
                    t.wait_ge(s_act, qj + 1)       # zsb[qj%NZ] written by copy1 of qj
                    if qj >= NY:
                        t.wait_ge(s_dve, qj - NY + 1)  # yps[qj%NY] freed by copy2 of qj-NY
                    for q in range(nqj):
                        i = nc.tensor.matmul(yps[qj % NY][:, q, :],
                                             zsb[qj % NZ][:, q, :], w64[:, :],
                                             start=(q % 4 == 0),
                                             stop=(q % 4 == 3 or q == nqj - 1))
                    i.then_inc(s_pe2, 1)

        @blk.scalar
        def _(a):
            for qi, (si, q0, nq) in enumerate(quads):
                a.wait_ge(s_pe1, qi + 1)
                if qi >= NZ:
                    a.wait_ge(s_pe2, qi - NZ + 1)  # zsb[qi%NZ] read done by mm2 of qi-NZ
                nc.scalar.copy(zsb[qi % NZ][:, 0:nq, 0:108],
                               zps[qi % NZ][:, 0:nq, 0:108]).then_inc(s_act, 1)

        @blk.vector
        def _(v):
            for qi, (si, q0, nq) in enumerate(quads):
                v.wait_ge(s_pe2, qi + 1)
                if si >= YR and q0 == 0:
                    v.wait_ge(s_out, 16 * (si - YR + 1))  # ytile reuse
                nc.vector.tensor_scalar_mul(
                    yt[si % YR][:, q0:q0 + nq, :],
                    yps[qi % NY][0:108, 0:nq, :], 1.0 / 48.0).then_inc(s_dve, 1)

        @blk.sync
        def _(s):
            # weight loads on the idle HWDGE ring so x loads start at t0;
            # the duplicate w36 load gives the completion slack.
            s.dma_start(w36[:, :], w36_d[:, :]).then_inc(s_w, 16)
            s.dma_start(w64[:, :], w64_d[:, :]).then_inc(s_w, 16)
            s.dma_start(w36[:, :], w36_d[:, :]).then_inc(s_w, 16)
    return nc


def _pack_x(xr, bases):
    """[ntok, D] fp32 -> [108, NG*128] bf16 in (t3,j | g,trip,c) layout."""
    import ml_dtypes
    ng = len(bases)
    # gather tokens: [ng, 6] token indices
    idx = np.asarray(bases)[:, None] + np.arange(6)[None, :]
    t = xr[idx.reshape(-1)]                       # [ng*6, D]
    t = t.reshape(ng, 2, 3, 36, 64)               # g, trip, t3, j, c
    t = t.transpose(2, 3, 0, 1, 4)                # t3, j, g, trip, c
    return np.ascontiguousarray(t.reshape(108, ng * 128)).astype(ml_dtypes.bfloat16)


def _unpack_y(yp, bases, ntok):
    """[108, NG*128] bf16 -> [ntok, D] fp32."""
    ng = len(bases)
    t = np.asarray(yp).astype(np.float32).reshape(3, 36, ng, 2, 64)  # t3,k,g,trip,m
    t = t.transpose(2, 3, 0, 1, 4).reshape(ng, 6, D)                 # g, (trip,t3)->tok, d
    out = np.empty((ntok, D), dtype=np.float32)
    nfull = ntok // 6
    out[: 6 * nfull] = t[:nfull].reshape(-1, D)
    if ntok % 6:
        out[ntok - 6:] = t[-1]
    return out


_CACHED = {}
_LAST_RES = None


def _run(x, had_k, ntok, ncores, trace=False):
    global _LAST_RES
    import ml_dtypes
    from concourse.bass_utils import run_bass_kernel_spmd

    h64 = _h64()
    w36_np = np.ascontiguousarray(
        np.kron(np.eye(3, dtype=np.float32), had_k.T.astype(np.float32)).astype(
            ml_dtypes.bfloat16
        )
    )
    w64_np = np.ascontiguousarray(
        np.kron(np.eye(2, dtype=np.float32), h64).astype(ml_dtypes.bfloat16)
    )

    bases = _group_bases(ntok)
    ng = len(bases)
    key = (ntok, w36_np.tobytes())
    if key not in _CACHED:
        _CACHED[key] = _build_program_raw(w36_np, w64_np, ng)
    nc = _CACHED[key]

    xf = np.ascontiguousarray(np.asarray(x, dtype=np.float32).reshape(-1, D))
    in_maps = [
        {"x": _pack_x(xf[i * ntok : (i + 1) * ntok], bases)} for i in range(ncores)
    ]
    res = run_bass_kernel_spmd(
        nc, in_maps, core_ids=list(range(ncores)), trace=trace
    )
    _LAST_RES = res
    y = np.concatenate(
        [_unpack_y(r["y"], bases, ntok) for r in res.results], axis=0
    )
    return y.reshape(x.shape)


def kernel(x, had_k):
    return _run(x, had_k, NTOK, NCORES)


# revision 56
# speedup vs baseline: 1.2063x; 1.2063x over previous
"""Hadamard transform kernel for Trainium2 (8 NeuronCores, SPMD data parallel).

y = (1/48) * (H36 (x) H64) @ x_row  per token row, x: (4, 8192, 2304) fp32.

Math: view each row as X[j=36, c=64] (row-major).  Then
    y[k*64+m] = (1/48) * sum_j sum_c had_k[k,j] * H64[m,c] * X[j,c]
with H64 the natural-order Sylvester Hadamard (symmetric).

Device scheme (per 6-token "group", no on-chip transposes needed):
  mm1: lhsT = Xg[(t3,j)=108 part, (trip2,c)=128 free]   (x data as stationary)
       rhs  = W36 = blockdiag(had_k.T x3) [108,108]
       out  = Z[(trip2,c)=128, (t3,k)=108]  (PSUM fp32)
  mm2: lhsT = Z (cast bf16) [128, 108]
       rhs  = W64 = blockdiag(H64 x2) [128,128]
       out  = Y[(t3,k)=108, (trip2,m)=128]  (PSUM fp32)

HBM layout: the HOST pre-permutes x into xp[108, NG*128] (partition-major,
per-group contiguous) and un-permutes y afterwards.  On device every DMA is
then one 2-4KB contiguous run per partition -- ~40x fewer descriptors than
the strided token layout, which puts the DMA path at HBM line rate.
"""

import numpy as np

D = 2304
NTOK = 4096          # tokens per core
NCORES = 8
SB_G = 15            # groups per superblock: 15*256B = 3840B per partition
                     # per DMA descriptor -- fits one <=4KB SDMA packet
QUAD = 8             # groups per PSUM bank batch


def _h64():
    m, c = np.meshgrid(np.arange(64), np.arange(64), indexing="ij")
    bits = np.zeros((64, 64), np.int64)
    v = m & c
    for _ in range(6):
        bits += v & 1
        v >>= 1
    return np.where(bits % 2 == 0, 1.0, -1.0).astype(np.float32)


def _group_bases(ntok):
    ngfull = ntok // 6
    bases = [6 * g for g in range(ngfull)]
    if ntok % 6:
        bases.append(ntok - 6)  # overlap group, rewrites a few tokens identically
    return bases


def _build_program_raw(w36_np, w64_np, ng_total):
    from contextlib import ExitStack
    import concourse.bass as bass
    import concourse.mybir as mybir
    from concourse.bass_types import AP

    FREE = ng_total * 128
    nc = bass.Bass()
    x = nc.dram_tensor("x", [108, FREE], mybir.dt.bfloat16, kind="ExternalInput")
    # y padded to 128 partitions: a 128-partition store spreads over all 16
    # SDMA engines (a 108-partition one only engages 12).  Rows 108:127 are
    # junk the host drops.
    y = nc.dram_tensor("y", [128, FREE], mybir.dt.bfloat16, kind="ExternalOutput")
    w36_d = nc.inline_tensor(w36_np, name="w36")
    w64_d = nc.inline_tensor(w64_np, name="w64")

    # superblocks: (first_group, n_groups)
    sbs = []
    g = 0
    while g < ng_total:
        n = min(SB_G, ng_total - g)
        sbs.append((g, n))
        g += n
    nsb = len(sbs)

    def dram_ap(t, g0, gcount, nparts=108):
        # one contiguous gcount*256B run per partition
        return AP(tensor=t, offset=g0 * 128,
                  ap=[[FREE, nparts], [1, gcount * 128]])

    # quads: global list of (sb_idx, q0, nq)
    quads = []
    for si, (g0, ng) in enumerate(sbs):
        q0 = 0
        while q0 < ng:
            quads.append((si, q0, min(QUAD, ng - q0)))
            q0 += QUAD
    nquads = len(quads)
    first_quad = [0]
    for si, (g0, ng) in enumerate(sbs):
        first_quad.append(first_quad[-1] + (ng + QUAD - 1) // QUAD)

    NZ = 2   # zps/zsb ring depth
    NY = 2   # yps ring depth

    with ExitStack() as ctx:
        w36 = ctx.enter_context(nc.sbuf_tensor("w36sb", [108, 108], mybir.dt.bfloat16))
        w64 = ctx.enter_context(nc.sbuf_tensor("w64sb", [128, 128], mybir.dt.bfloat16))

        XR = 6   # xt ring: loads run several sbs ahead of the PE
        YR = 3
        xt = [ctx.enter_context(nc.sbuf_tensor(f"xt{i}", [108, SB_G, 128], mybir.dt.bfloat16)) for i in range(XR)]
        yt = [ctx.enter_context(nc.sbuf_tensor(f"yt{i}", [128, SB_G, 128], mybir.dt.bfloat16)) for i in range(YR)]
        # zsb/yps padded to 128 free columns / partitions: a full-128-column
        # stationary makes mm2's LDWEIGHTS eligible for fast weight load.
        # Columns 108:128 hold stale garbage that lands in PSUM partitions
        # 108:128, which nothing reads.
        # zps group stride padded to 128 (512B, bank-aligned) so no matmul
        # output tile straddles a 2KB PSUM bank boundary.
        zsb = [ctx.enter_context(nc.sbuf_tensor(f"zsb{i}", [128, QUAD, 128], mybir.dt.bfloat16)) for i in range(NZ)]
        zps = [ctx.enter_context(nc.psum_tensor(f"zps{i}", [128, QUAD, 128], mybir.dt.float32)) for i in range(NZ)]
        yps = [ctx.enter_context(nc.psum_tensor(f"yps{i}", [128, QUAD, 128], mybir.dt.float32)) for i in range(NY)]
        s_in = ctx.enter_context(nc.semaphore())
        s_pe1 = ctx.enter_context(nc.semaphore())
        s_act = ctx.enter_context(nc.semaphore())
        s_pe2 = ctx.enter_context(nc.semaphore())
        s_dve = ctx.enter_context(nc.semaphore())
        s_out = ctx.enter_context(nc.semaphore())
        s_w = ctx.enter_context(nc.semaphore())
        s_nil = ctx.enter_context(nc.semaphore())  # sink for data DMAs; never waited on
        blk = ctx.enter_context(nc.Block())

        @blk.gpsimd
        def _(g):
            # Loads AND stores share the SWDGE queue (spreads packets over
            # all 16 SDMA engines; HWDGE only uses 12 for this shape).
            # Stores trail the loads by 5 sbs in the issue stream so the
            # store's data-ready wait never delays a load that keeps the
            # PE fed.
            # A DMA's 16 sem incs are NOT strictly ordered after all of its
            # data descriptors when the transfer is only ~108 descriptors,
            # so consumers wait for the NEXT transfer's incs too (one sb of
            # slack >> the observed sub-us race window); a dummy trailing
            # load provides that slack for the final sb.
            g.dma_start(w36[:, :], w36_d[:, :]).then_inc(s_w, 16)
            g.dma_start(w64[:, :], w64_d[:, :]).then_inc(s_w, 16)
            for si, (g0, ng) in enumerate(sbs):
                if si >= XR:  # xtile reuse: mm1s of sb si-XR done
                    g.wait_ge(s_pe1, first_quad[si - XR + 1])
                g.dma_start(xt[si % XR][:, 0:ng, :],
                            dram_ap(x, g0, ng)).then_inc(s_in, 16)
            g.wait_ge(s_pe1, first_quad[nsb - XR + 1])  # xt[nsb%XR] free
            g.dma_start(xt[nsb % XR][:, 0:1, :],
                        dram_ap(x, 0, 1)).then_inc(s_in, 16)

        @blk.tensor
        def _(t):
            # Software-pipelined: mm1 of quad qi runs ahead of mm2 of quad
            # qi-1, so the scalar copy1(qi-1) overlaps mm1(qi) instead of
            # stalling the PE.
            t.wait_ge(s_w, 32)
            for qi in range(nquads + 1):
                if qi < nquads:
                    si, q0, nq = quads[qi]
                    if q0 == 0:
                        t.wait_ge(s_in, 16 * (si + 2))  # sb si + one-sb slack
                    # (zps[qi%NZ] free is implied: the previous iteration's
                    # mm2 wait saw s_act >= qi, and copy1 is monotone)
                    for q in range(nq):
                        i = nc.tensor.matmul(zps[qi % NZ][:, q, 0:108],
                                             xt[si % XR][:, q0 + q, :], w36[:, :],
                                             start=(q % 4 == 0),
                                             stop=(q % 4 == 3 or q == nq - 1))
                    i.then_inc(s_pe1, 1)
                if qi >= 1:
                    qj = qi - 1
                    _, _, nqj = quads[qj]
                    t.wait_ge(s_act, qj + 1)       # zsb[qj%NZ] written by copy1 of qj
                    if qj >= NY:
                        t.wait_ge(s_dve, qj - NY + 1)  # yps[qj%NY] freed by copy2 of qj-NY
                    for q in range(nqj):
                        i = nc.tensor.matmul(yps[qj % NY][:, q, :],
                                             zsb[qj % NZ][:, q, :], w64[:, :],
                                             start=(q % 4 == 0),
                                             stop=(q % 4 == 3 or q == nqj - 1))
                    i.then_inc(s_pe2, 1)

        @blk.scalar
        def _(a):
            for qi, (si, q0, nq) in enumerate(quads):
                a.wait_ge(s_pe1, qi + 1)
                if qi >= NZ:
                    a.wait_ge(s_pe2, qi - NZ + 1)  # zsb[qi%NZ] read done by mm2 of qi-NZ
                nc.scalar.copy(zsb[qi % NZ][:, 0:nq, 0:108],
                               zps[qi % NZ][:, 0:nq, 0:108]).then_inc(s_act, 1)

        @blk.vector
        def _(v):
            for qi, (si, q0, nq) in enumerate(quads):
                v.wait_ge(s_pe2, qi + 1)
                if si >= YR and q0 == 0:
                    v.wait_ge(s_out, 16 * (si - YR + 1))  # ytile reuse
                nc.vector.tensor_scalar_mul(
                    yt[si % YR][0:108, q0:q0 + nq, :],
                    yps[qi % NY][0:108, 0:nq, :], 1.0 / 48.0).then_inc(s_dve, 1)

        @blk.sync
        def _(s):
            for si, (g0, ng) in enumerate(sbs):
                s.wait_ge(s_dve, first_quad[si + 1])
                s.dma_start(dram_ap(y, g0, ng, nparts=128),
                            yt[si % YR][:, 0:ng, :]).then_inc(s_out, 16)
    return nc


def _pack_x(xr, bases):
    """[ntok, D] fp32 -> [108, NG*128] bf16 in (t3,j | g,trip,c) layout."""
    import ml_dtypes
    ng = len(bases)
    # gather tokens: [ng, 6] token indices
    idx = np.asarray(bases)[:, None] + np.arange(6)[None, :]
    t = xr[idx.reshape(-1)]                       # [ng*6, D]
    t = t.reshape(ng, 2, 3, 36, 64)               # g, trip, t3, j, c
    t = t.transpose(2, 3, 0, 1, 4)                # t3, j, g, trip, c
    return np.ascontiguousarray(t.reshape(108, ng * 128)).astype(ml_dtypes.bfloat16)


def _unpack_y(yp, bases, ntok):
    """[128, NG*128] bf16 (rows 108: junk) -> [ntok, D] fp32."""
    ng = len(bases)
    t = np.asarray(yp)[0:108].astype(np.float32).reshape(3, 36, ng, 2, 64)  # t3,k,g,trip,m
    t = t.transpose(2, 3, 0, 1, 4).reshape(ng, 6, D)                 # g, (trip,t3)->tok, d
    out = np.empty((ntok, D), dtype=np.float32)
    nfull = ntok // 6
    out[: 6 * nfull] = t[:nfull].reshape(-1, D)
    if ntok % 6:
        out[ntok - 6:] = t[-1]
    return out


_CACHED = {}
_LAST_RES = None


def _run(x, had_k, ntok, ncores, trace=False):
    global _LAST_RES
    import ml_dtypes
    from concourse.bass_utils import run_bass_kernel_spmd

    h64 = _h64()
    w36_np = np.ascontiguousarray(
        np.kron(np.eye(3, dtype=np.float32), had_k.T.astype(np.float32)).astype(
            ml_dtypes.bfloat16
        )
    )
    w64_np = np.ascontiguousarray(
        np.kron(np.eye(2, dtype=np.float32), h64).astype(ml_dtypes.bfloat16)
    )

    bases = _group_bases(ntok)
    ng = len(bases)
    key = (ntok, w36_np.tobytes())
    if key not in _CACHED:
        _CACHED[key] = _build_program_raw(w36_np, w64_np, ng)
    nc = _CACHED[key]

    xf = np.ascontiguousarray(np.asarray(x, dtype=np.float32).reshape(-1, D))
    in_maps = [
        {"x": _pack_x(xf[i * ntok : (i + 1) * ntok], bases)} for i in range(ncores)
    ]
    res = run_bass_kernel_spmd(
        nc, in_maps, core_ids=list(range(ncores)), trace=trace
    )
    _LAST_RES = res
    y = np.concatenate(
        [_unpack_y(r["y"], bases, ntok) for r in res.results], axis=0
    )
    return y.reshape(x.shape)


def kernel(x, had_k):
    return _run(x, had_k, NTOK, NCORES)


# revision 58
# speedup vs baseline: 1.2112x; 1.0041x over previous
"""Hadamard transform kernel for Trainium2 (8 NeuronCores, SPMD data parallel).

y = (1/48) * (H36 (x) H64) @ x_row  per token row, x: (4, 8192, 2304) fp32.

Math: view each row as X[j=36, c=64] (row-major).  Then
    y[k*64+m] = (1/48) * sum_j sum_c had_k[k,j] * H64[m,c] * X[j,c]
with H64 the natural-order Sylvester Hadamard (symmetric).

Device scheme (per 6-token "group", no on-chip transposes needed):
  mm1: lhsT = Xg[(t3,j)=108 part, (trip2,c)=128 free]   (x data as stationary)
       rhs  = W36 = blockdiag(had_k.T x3) [108,108]
       out  = Z[(trip2,c)=128, (t3,k)=108]  (PSUM fp32)
  mm2: lhsT = Z (cast bf16) [128, 108]
       rhs  = W64 = blockdiag(H64 x2) [128,128]
       out  = Y[(t3,k)=108, (trip2,m)=128]  (PSUM fp32)

HBM layout: the HOST pre-permutes x into xp[108, NG*128] (partition-major,
per-group contiguous) and un-permutes y afterwards.  On device every DMA is
then one 2-4KB contiguous run per partition -- ~40x fewer descriptors than
the strided token layout, which puts the DMA path at HBM line rate.
"""

import numpy as np

D = 2304
NTOK = 4096          # tokens per core
NCORES = 8
SB_G = 15            # groups per superblock: 15*256B = 3840B per partition
                     # per DMA descriptor -- fits one <=4KB SDMA packet
QUAD = 8             # groups per PSUM bank batch


def _h64():
    m, c = np.meshgrid(np.arange(64), np.arange(64), indexing="ij")
    bits = np.zeros((64, 64), np.int64)
    v = m & c
    for _ in range(6):
        bits += v & 1
        v >>= 1
    return np.where(bits % 2 == 0, 1.0, -1.0).astype(np.float32)


def _group_bases(ntok):
    ngfull = ntok // 6
    bases = [6 * g for g in range(ngfull)]
    if ntok % 6:
        bases.append(ntok - 6)  # overlap group, rewrites a few tokens identically
    return bases


def _build_program_raw(w36_np, w64_np, ng_total):
    from contextlib import ExitStack
    import concourse.bass as bass
    import concourse.mybir as mybir
    from concourse.bass_types import AP

    FREE = ng_total * 128
    nc = bass.Bass()
    x = nc.dram_tensor("x", [108, FREE], mybir.dt.bfloat16, kind="ExternalInput")
    # y padded to 128 partitions: a 128-partition store spreads over all 16
    # SDMA engines (a 108-partition one only engages 12).  Rows 108:127 are
    # junk the host drops.
    y = nc.dram_tensor("y", [128, FREE], mybir.dt.bfloat16, kind="ExternalOutput")
    w36_d = nc.inline_tensor(w36_np, name="w36")
    w64_d = nc.inline_tensor(w64_np, name="w64")

    # superblocks: (first_group, n_groups)
    sbs = []
    g = 0
    while g < ng_total:
        n = min(SB_G, ng_total - g)
        sbs.append((g, n))
        g += n
    nsb = len(sbs)

    def dram_ap(t, g0, gcount, nparts=108):
        # one contiguous gcount*256B run per partition
        return AP(tensor=t, offset=g0 * 128,
                  ap=[[FREE, nparts], [1, gcount * 128]])

    # quads: global list of (sb_idx, q0, nq)
    quads = []
    for si, (g0, ng) in enumerate(sbs):
        q0 = 0
        while q0 < ng:
            quads.append((si, q0, min(QUAD, ng - q0)))
            q0 += QUAD
    nquads = len(quads)
    first_quad = [0]
    for si, (g0, ng) in enumerate(sbs):
        first_quad.append(first_quad[-1] + (ng + QUAD - 1) // QUAD)

    NZ = 2   # zps/zsb ring depth
    NY = 2   # yps ring depth

    with ExitStack() as ctx:
        w36 = ctx.enter_context(nc.sbuf_tensor("w36sb", [108, 108], mybir.dt.bfloat16))
        w64 = ctx.enter_context(nc.sbuf_tensor("w64sb", [128, 128], mybir.dt.bfloat16))

        XR = 6   # xt ring: loads run several sbs ahead of the PE
        YR = 3
        xt = [ctx.enter_context(nc.sbuf_tensor(f"xt{i}", [108, SB_G, 128], mybir.dt.bfloat16)) for i in range(XR)]
        yt = [ctx.enter_context(nc.sbuf_tensor(f"yt{i}", [128, SB_G, 128], mybir.dt.bfloat16)) for i in range(YR)]
        # zsb/yps padded to 128 free columns / partitions: a full-128-column
        # stationary makes mm2's LDWEIGHTS eligible for fast weight load.
        # Columns 108:128 hold stale garbage that lands in PSUM partitions
        # 108:128, which nothing reads.
        # zps group stride padded to 128 (512B, bank-aligned) so no matmul
        # output tile straddles a 2KB PSUM bank boundary.
        zsb = [ctx.enter_context(nc.sbuf_tensor(f"zsb{i}", [128, QUAD, 128], mybir.dt.bfloat16)) for i in range(NZ)]
        zps = [ctx.enter_context(nc.psum_tensor(f"zps{i}", [128, QUAD, 128], mybir.dt.float32)) for i in range(NZ)]
        yps = [ctx.enter_context(nc.psum_tensor(f"yps{i}", [128, QUAD, 128], mybir.dt.float32)) for i in range(NY)]
        s_in = ctx.enter_context(nc.semaphore())
        s_pe1 = ctx.enter_context(nc.semaphore())
        s_act = ctx.enter_context(nc.semaphore())
        s_pe2 = ctx.enter_context(nc.semaphore())
        s_dve = ctx.enter_context(nc.semaphore())
        s_out = ctx.enter_context(nc.semaphore())
        s_w = ctx.enter_context(nc.semaphore())
        s_nil = ctx.enter_context(nc.semaphore())  # sink for data DMAs; never waited on
        blk = ctx.enter_context(nc.Block())

        @blk.gpsimd
        def _(g):
            # Loads AND stores share the SWDGE queue (spreads packets over
            # all 16 SDMA engines; HWDGE only uses 12 for this shape).
            # Stores trail the loads by 5 sbs in the issue stream so the
            # store's data-ready wait never delays a load that keeps the
            # PE fed.
            # A DMA's 16 sem incs are NOT strictly ordered after all of its
            # data descriptors when the transfer is only ~108 descriptors,
            # so consumers wait for the NEXT transfer's incs too (one sb of
            # slack >> the observed sub-us race window); a dummy trailing
            # load provides that slack for the final sb.
            for si, (g0, ng) in enumerate(sbs):
                if si >= XR:  # xtile reuse: mm1s of sb si-XR done
                    g.wait_ge(s_pe1, first_quad[si - XR + 1])
                g.dma_start(xt[si % XR][:, 0:ng, :],
                            dram_ap(x, g0, ng)).then_inc(s_in, 16)
            g.wait_ge(s_pe1, first_quad[nsb - XR + 1])  # xt[nsb%XR] free
            g.dma_start(xt[nsb % XR][:, 0:1, :],
                        dram_ap(x, 0, 1)).then_inc(s_in, 16)

        @blk.tensor
        def _(t):
            # Software-pipelined: mm1 of quad qi runs ahead of mm2 of quad
            # qi-1, so the scalar copy1(qi-1) overlaps mm1(qi) instead of
            # stalling the PE.
            t.wait_ge(s_w, 32)
            for qi in range(nquads + 1):
                if qi < nquads:
                    si, q0, nq = quads[qi]
                    if q0 == 0:
                        t.wait_ge(s_in, 16 * (si + 2))  # sb si + one-sb slack
                    # (zps[qi%NZ] free is implied: the previous iteration's
                    # mm2 wait saw s_act >= qi, and copy1 is monotone)
                    for q in range(nq):
                        i = nc.tensor.matmul(zps[qi % NZ][:, q, 0:108],
                                             xt[si % XR][:, q0 + q, :], w36[:, :],
                                             start=(q % 4 == 0),
                                             stop=(q % 4 == 3 or q == nq - 1))
                    i.then_inc(s_pe1, 1)
                if qi >= 1:
                    qj = qi - 1
                    _, _, nqj = quads[qj]
                    t.wait_ge(s_act, qj + 1)       # zsb[qj%NZ] written by copy1 of qj
                    if qj >= NY:
                        t.wait_ge(s_dve, qj - NY + 1)  # yps[qj%NY] freed by copy2 of qj-NY
                    for q in range(nqj):
                        i = nc.tensor.matmul(yps[qj % NY][:, q, :],
                                             zsb[qj % NZ][:, q, :], w64[:, :],
                                             start=(q % 4 == 0),
                                             stop=(q % 4 == 3 or q == nqj - 1))
                    i.then_inc(s_pe2, 1)

        @blk.scalar
        def _(a):
            # weight loads ride the otherwise-idle ACT HWDGE ring so the x
            # loads on gpsimd start at t0 (the tensor engine's later s_in
            # wait provides ~5us of completion slack for these).
            nc.scalar.dma_start(w36[:, :], w36_d[:, :]).then_inc(s_w, 16)
            nc.scalar.dma_start(w64[:, :], w64_d[:, :]).then_inc(s_w, 16)
            for qi, (si, q0, nq) in enumerate(quads):
                a.wait_ge(s_pe1, qi + 1)
                if qi >= NZ:
                    a.wait_ge(s_pe2, qi - NZ + 1)  # zsb[qi%NZ] read done by mm2 of qi-NZ
                nc.scalar.copy(zsb[qi % NZ][:, 0:nq, 0:108],
                               zps[qi % NZ][:, 0:nq, 0:108]).then_inc(s_act, 1)

        @blk.vector
        def _(v):
            for qi, (si, q0, nq) in enumerate(quads):
                v.wait_ge(s_pe2, qi + 1)
                if si >= YR and q0 == 0:
                    v.wait_ge(s_out, 16 * (si - YR + 1))  # ytile reuse
                nc.vector.tensor_scalar_mul(
                    yt[si % YR][0:108, q0:q0 + nq, :],
                    yps[qi % NY][0:108, 0:nq, :], 1.0 / 48.0).then_inc(s_dve, 1)

        @blk.sync
        def _(s):
            for si, (g0, ng) in enumerate(sbs):
                s.wait_ge(s_dve, first_quad[si + 1])
                s.dma_start(dram_ap(y, g0, ng, nparts=128),
                            yt[si % YR][:, 0:ng, :]).then_inc(s_out, 16)
    return nc


def _pack_x(xr, bases):
    """[ntok, D] fp32 -> [108, NG*128] bf16 in (t3,j | g,trip,c) layout."""
    import ml_dtypes
    ng = len(bases)
    # gather tokens: [ng, 6] token indices
    idx = np.asarray(bases)[:, None] + np.arange(6)[None, :]
    t = xr[idx.reshape(-1)]                       # [ng*6, D]
    t = t.reshape(ng, 2, 3, 36, 64)               # g, trip, t3, j, c
    t = t.transpose(2, 3, 0, 1, 4)                # t3, j, g, trip, c
    return np.ascontiguousarray(t.reshape(108, ng * 128)).astype(ml_dtypes.bfloat16)


def _unpack_y(yp, bases, ntok):
    """[128, NG*128] bf16 (rows 108: junk) -> [ntok, D] fp32."""
    ng = len(bases)
    t = np.asarray(yp)[0:108].astype(np.float32).reshape(3, 36, ng, 2, 64)  # t3,k,g,trip,m
    t = t.transpose(2, 3, 0, 1, 4).reshape(ng, 6, D)                 # g, (trip,t3)->tok, d
    out = np.empty((ntok, D), dtype=np.float32)
    nfull = ntok // 6
    out[: 6 * nfull] = t[:nfull].reshape(-1, D)
    if ntok % 6:
        out[ntok - 6:] = t[-1]
    return out


_CACHED = {}
_LAST_RES = None


def _run(x, had_k, ntok, ncores, trace=False):
    global _LAST_RES
    import ml_dtypes
    from concourse.bass_utils import run_bass_kernel_spmd

    h64 = _h64()
    w36_np = np.ascontiguousarray(
        np.kron(np.eye(3, dtype=np.float32), had_k.T.astype(np.float32)).astype(
            ml_dtypes.bfloat16
        )
    )
    w64_np = np.ascontiguousarray(
        np.kron(np.eye(2, dtype=np.float32), h64).astype(ml_dtypes.bfloat16)
    )

    bases = _group_bases(ntok)
    ng = len(bases)
    key = (ntok, w36_np.tobytes())
    if key not in _CACHED:
        _CACHED[key] = _build_program_raw(w36_np, w64_np, ng)
    nc = _CACHED[key]

    xf = np.ascontiguousarray(np.asarray(x, dtype=np.float32).reshape(-1, D))
    in_maps = [
        {"x": _pack_x(xf[i * ntok : (i + 1) * ntok], bases)} for i in range(ncores)
    ]
    res = run_bass_kernel_spmd(
        nc, in_maps, core_ids=list(range(ncores)), trace=trace
    )
    _LAST_RES = res
    y = np.concatenate(
        [_unpack_y(r["y"], bases, ntok) for r in res.results], axis=0
    )
    return y.reshape(x.shape)


def kernel(x, had_k):
    return _run(x, had_k, NTOK, NCORES)
